# revision 1
# baseline (speedup 1.0000x reference)
"""Trainium2 Bass kernel for nn_CrossProduct (factorization-machine cross term).

out = 0.5 * sum_n [(x @ v)^2 - (x^2) @ (v^2)]   per row, shape (B, 1)

Math restructuring:
  sum_n (x^2 @ v^2)[b, n] = x_b^2 . w   with w = rowsum(v^2)  (1024,)
  => out_b = sum_n (x_b @ (v/sqrt2))^2  +  x_b^2 . (-0.5 w)

Distribution: pure data-parallel over batch across 8 NeuronCores
(2048 rows/core); vparam-derived weights replicated.

Per-core device program (bf16 inputs, fp32 PSUM accumulation):
  - x shipped pre-transposed/chunked from host as XT[p, m, c, b'] =
    x[m*512+b', c*128+p] in bf16 (k on partitions -> natural matmul lhs/rhs).
  - per b-tile m (4 x 512 cols):
      * square xt on DVE+GpSimd -> x2 (bf16)
      * PE: psumA[64,512]   += v_c'.T @ xt_c      (term 1, cols 0-63 of PE)
            psumO[64:65,:]  += wneg_c.T @ x2_c    (term 2, col 64, concurrent)
      * DVE: sq = psumA^2 (fp32)
      * PE: psumO[64:65,:] += ones.T @ sq  (f32r, 1 cyc/row)  == final out row
      * ACT: copy psumO -> out SBUF row
  - single 8KB DMA of the 2048 outputs.
"""

import math
from contextlib import ExitStack

import ml_dtypes
import numpy as np

import concourse.bass as bass
import concourse.bacc as bacc
import concourse.mybir as mybir
import concourse.tile as tile
from concourse.bass_utils import run_bass_kernel_spmd

BF16 = mybir.dt.bfloat16
F32 = mybir.dt.float32
F32R = mybir.dt.float32r

N_CORES = 8
B, XD, KD = 16384, 1024, 64
BS = B // N_CORES  # 2048 batch rows per core
C = XD // 128      # 8 contraction chunks of 128
MT = 4             # b-tiles per core
BT = BS // MT      # 512 batch cols per tile
DVE_CHUNKS = 5     # chunks squared on DVE; rest on GpSimd


def _body(ctx, tc, OUT, XT, VW):
    nc = tc.nc
    const = ctx.enter_context(tc.tile_pool(name="const", bufs=1))
    xpool = ctx.enter_context(tc.tile_pool(name="xp", bufs=4))
    x2apool = ctx.enter_context(tc.tile_pool(name="x2a", bufs=4))
    x2bpool = ctx.enter_context(tc.tile_pool(name="x2b", bufs=4))
    sqpool = ctx.enter_context(tc.tile_pool(name="sqp", bufs=4))
    opool = ctx.enter_context(tc.tile_pool(name="op", bufs=1))
    psA = ctx.enter_context(tc.tile_pool(name="psA", bufs=4, space="PSUM"))
    psO = ctx.enter_context(tc.tile_pool(name="psO", bufs=4, space="PSUM"))

    # vw columns per chunk c: [0:64]=v/sqrt2, 64=-0.5*w, 65=1.0 (reduce
    # weights), 66=0.0 (activation bias source) -- one DMA, so every PE
    # weight load is covered by a single already-observed semaphore.
    vw = const.tile([128, C, 67], BF16)
    nc.scalar.dma_start(vw[:], VW)
    outs = opool.tile([65, BS], F32)
    # one-time ACT touch of the vw DMA so later Square ops (which read the
    # bias column) carry only their PE wait (1-wait ISA limit per inst).
    actwarm = const.tile([128, 1], BF16)
    nc.scalar.copy(actwarm[:], vw[:, 0, 66:67])

    for m in range(MT):
        xt = xpool.tile([128, C, BT], BF16)
        # alternate the two HWDGE rings (SP / ACT) to double load bandwidth
        (nc.sync if m % 2 == 0 else nc.scalar).dma_start(xt[:], XT[:, m])
        x2a = x2apool.tile([128, DVE_CHUNKS, BT], BF16)
        nc.vector.tensor_mul(
            x2a[:], xt[:, 0:DVE_CHUNKS], xt[:, 0:DVE_CHUNKS]
        )
        x2b = x2bpool.tile([128, C - DVE_CHUNKS, BT], BF16)
        nc.gpsimd.tensor_mul(
            x2b[:], xt[:, DVE_CHUNKS:C], xt[:, DVE_CHUNKS:C]
        )

        pa = psA.tile([64, BT], F32)
        po = psO.tile([65, BT], F32)
        for c in range(C):
            nc.tensor.matmul(
                pa[:],
                vw[:, c, 0:64],
                xt[:, c],
                start=(c == 0),
                stop=(c == C - 1),
                tile_position=(0, 0),
            )
            nc.tensor.matmul(
                po[64:65, :],
                vw[:, c, 64:65],
                x2a[:, c] if c < DVE_CHUNKS else x2b[:, c - DVE_CHUNKS],
                start=(c == 0),
                stop=False,
                tile_position=(0, 64),
            )
        sq = sqpool.tile([64, BT], BF16)
        nc.scalar.activation(
            sq[:],
            pa[:],
            mybir.ActivationFunctionType.Square,
            bias=vw[0:64, 0, 66:67],
        )
        nc.tensor.matmul(
            po[64:65, :],
            vw[0:64, 0, 65:66],
            sq[:],
            start=False,
            stop=True,
            tile_position=(0, 64),
        )
        nc.scalar.copy(outs[64:65, m * BT : (m + 1) * BT], po[64:65, :])

    nc.sync.dma_start(OUT, outs[64:65, :])


_NC_CACHE = None


def build_nc():
    global _NC_CACHE
    if _NC_CACHE is not None:
        return _NC_CACHE
    nc = bacc.Bacc("TRN2", target_bir_lowering=False, debug=False)
    XT = nc.dram_tensor("XT", [128, MT, C, BT], BF16, kind="ExternalInput").ap()
    VW = nc.dram_tensor("VW", [128, C, 67], BF16, kind="ExternalInput").ap()
    OUT = nc.dram_tensor("OUT", [1, BS], F32, kind="ExternalOutput").ap()
    with tile.TileContext(nc) as tc:
        with ExitStack() as ctx:
            _body(ctx, tc, OUT, XT, VW)
    nc.compile()
    _NC_CACHE = nc
    return nc


def make_in_maps(x, vparam):
    bf = ml_dtypes.bfloat16
    x = np.ascontiguousarray(x, dtype=np.float32)
    v = np.ascontiguousarray(vparam, dtype=np.float32)

    vs = (v / math.sqrt(2.0)).astype(bf)             # (1024, 64)
    w = (v.astype(np.float64) ** 2).sum(axis=1)
    wneg = (-0.5 * w).astype(np.float32).astype(bf)  # (1024,)

    VWh = np.empty((128, C, 67), dtype=bf)
    VWh[:, :, 0:64] = vs.reshape(C, 128, KD).transpose(1, 0, 2)
    VWh[:, :, 64] = wneg.reshape(C, 128).T
    VWh[:, :, 65] = bf(1.0)
    VWh[:, :, 66] = bf(0.0)

    in_maps = []
    for i in range(N_CORES):
        xs = x[i * BS : (i + 1) * BS]                # (2048, 1024)
        xt = np.ascontiguousarray(xs.T)              # (1024, 2048) [k, b]
        # A[p, m, c, b'] = xt[c*128+p, m*512+b']
        A = xt.reshape(C, 128, MT, BT).transpose(1, 2, 0, 3)
        XTh = np.ascontiguousarray(A).astype(bf)
        in_maps.append({"XT": XTh, "VW": VWh})
    return in_maps


LAST_RESULTS = None  # stashed BassKernelResults (for test harness profiling)
TRACE = False


def kernel(x, vparam):
    global LAST_RESULTS
    nc = build_nc()
    in_maps = make_in_maps(x, vparam)
    res = run_bass_kernel_spmd(nc, in_maps, list(range(N_CORES)), trace=TRACE)
    LAST_RESULTS = res
    out = np.concatenate(
        [res.results[i]["OUT"].reshape(BS, 1) for i in range(N_CORES)], axis=0
    )
    return out.astype(np.float32)



# revision 3
# speedup vs baseline: 1.1928x; 1.1928x over previous
"""Trainium2 Bass kernel for nn_CrossProduct (factorization-machine cross term).

out_b = 0.5 * [ ||x_b V||^2 - sum_n w_n x_bn^2 ],  w = rowsum(V^2)

Math restructuring (v2): ship xu = 4*(x*sqrt(w)) in bf16. Then
  term2_b = sum_n xu_bn^2 / 32          (plain square-sum, no weights)
  term1   = sum_k (xu_b @ V')^2,  V' = v/(4*u*sqrt(2))
so the PE never streams a weighted x^2 matmul:
  - 8 bf16 matmuls/tile: psumA[64,512] += V'[c].T @ xu[c]
  - squares xu^2 -> fp8e4m3 on ACT(2 chunks)/DVE(4)/GpSimd(2)
  - 4 fp8 DoubleRow matmuls/tile (2 chunks per stream, const -2^-5 lhsT)
    accumulate -term2 into psumO[1,512]
  - ACT: sq = Square(psumA) -> bf16; 1 bf16 matmul adds ones.T @ sq
  - ACT copies psumO -> out row; single 8KB DMA at the end.

Distribution: pure data-parallel over batch across 8 NeuronCores
(2048 rows/core); vparam-derived constants replicated.
"""

import math
from contextlib import ExitStack

import ml_dtypes
import numpy as np

import concourse.bass as bass
import concourse.bacc as bacc
import concourse.mybir as mybir
import concourse.tile as tile
from concourse.bass_utils import run_bass_kernel_spmd

BF16 = mybir.dt.bfloat16
F32 = mybir.dt.float32
F8 = mybir.dt.float8e4

N_CORES = 8
B, XD, KD = 16384, 1024, 64
BS = B // N_CORES  # 2048 batch rows per core
C = XD // 128      # 8 contraction chunks of 128
MT = 4             # b-tiles per core
BT = BS // MT      # 512 batch cols per tile

DR = mybir.MatmulPerfMode.DoubleRow
SQUARE = mybir.ActivationFunctionType.Square


def _body(ctx, tc, OUT, XU, VW, VF8):
    nc = tc.nc
    const = ctx.enter_context(tc.tile_pool(name="const", bufs=1))
    xpool = ctx.enter_context(tc.tile_pool(name="xp", bufs=3))
    qpool = ctx.enter_context(tc.tile_pool(name="qp", bufs=2))
    sqpool = ctx.enter_context(tc.tile_pool(name="sqp", bufs=2))
    opool = ctx.enter_context(tc.tile_pool(name="op", bufs=1))
    psA = ctx.enter_context(tc.tile_pool(name="psA", bufs=2, space="PSUM"))
    psO = ctx.enter_context(tc.tile_pool(name="psO", bufs=2, space="PSUM"))

    # vw columns: [0:64]=V', 64=1.0 (sq-reduce weights), 65=0.0 (ACT bias)
    vw = const.tile([128, C, 66], BF16)
    nc.scalar.dma_start(vw[:], VW)
    vf8 = const.tile([128, 2, 32], F8)
    nc.scalar.dma_start(vf8[:], VF8)
    outs = opool.tile([1, BS], F32)
    # one-time ACT touch of the vw DMA so later Square ops (which read the
    # bias column) carry only their data-dependency wait.
    actwarm = const.tile([128, 1], BF16)
    nc.scalar.copy(actwarm[:], vw[:, 0, 65:66])

    for m in range(MT):
        xt = xpool.tile([128, C, BT], BF16)
        if m == 0:
            # fine-grained first-tile DMA so the PE starts ASAP
            for h in range(4):
                nc.sync.dma_start(xt[:, 2 * h : 2 * h + 2], XU[:, m, 2 * h : 2 * h + 2])
        else:
            nc.sync.dma_start(xt[:, 0:4], XU[:, m, 0:4])
            nc.sync.dma_start(xt[:, 4:8], XU[:, m, 4:8])

        # squares -> fp8, split across engines (chunk pairs align with the
        # DoubleRow rhs groups so each DR matmul has a single producer)
        xq = qpool.tile([128, C, BT], F8)
        nc.scalar.activation(xq[:, 0:2], xt[:, 0:2], SQUARE, bias=vw[:, 0, 65:66])
        nc.vector.tensor_mul(xq[:, 2:4], xt[:, 2:4], xt[:, 2:4])
        nc.vector.tensor_mul(xq[:, 4:6], xt[:, 4:6], xt[:, 4:6])
        nc.gpsimd.tensor_mul(xq[:, 6:8], xt[:, 6:8], xt[:, 6:8])

        pa = psA.tile([64, BT], F32)
        po = psO.tile([32, BT], F32)
        for c in range(C):
            nc.tensor.matmul(
                pa[:],
                vw[:, c, 0:64],
                xt[:, c],
                start=(c == 0),
                stop=(c == C - 1),
                tile_position=(0, 0),
            )
        for t in range(4):
            nc.tensor.matmul(
                po[:],
                vf8[:],
                xq[:, 2 * t : 2 * t + 2],
                start=(t == 0),
                stop=False,
                perf_mode=DR,
            )
        sq = sqpool.tile([64, BT], BF16)
        nc.scalar.activation(sq[:], pa[:], SQUARE, bias=vw[0:64, 0, 65:66])
        nc.tensor.matmul(
            po[0:1, :],
            vw[0:64, 0, 64:65],
            sq[:],
            start=False,
            stop=True,
        )
        nc.scalar.copy(outs[:, m * BT : (m + 1) * BT], po[0:1, :])

    nc.sync.dma_start(OUT, outs[:])


_NC_CACHE = None


def build_nc():
    global _NC_CACHE
    if _NC_CACHE is not None:
        return _NC_CACHE
    nc = bacc.Bacc("TRN2", target_bir_lowering=False, debug=False)
    XU = nc.dram_tensor("XU", [128, MT, C, BT], BF16, kind="ExternalInput").ap()
    VW = nc.dram_tensor("VW", [128, C, 66], BF16, kind="ExternalInput").ap()
    VF8 = nc.dram_tensor("VF8", [128, 2, 32], F8, kind="ExternalInput").ap()
    OUT = nc.dram_tensor("OUT", [1, BS], F32, kind="ExternalOutput").ap()
    with tile.TileContext(nc) as tc:
        with ExitStack() as ctx:
            _body(ctx, tc, OUT, XU, VW, VF8)
    nc.compile()
    _NC_CACHE = nc
    return nc


def make_in_maps(x, vparam):
    bf = ml_dtypes.bfloat16
    f8 = ml_dtypes.float8_e4m3
    x = np.ascontiguousarray(x, dtype=np.float32)
    v = np.ascontiguousarray(vparam, dtype=np.float64)

    w = (v**2).sum(axis=1)                      # (1024,)
    u = np.sqrt(w)
    vs = (v / (u[:, None] * 4.0 * math.sqrt(2.0))).astype(np.float32).astype(bf)
    xu = (x.astype(np.float64) * (4.0 * u)[None, :]).astype(np.float32)

    VWh = np.empty((128, C, 66), dtype=bf)
    VWh[:, :, 0:64] = np.asarray(vs).reshape(C, 128, KD).transpose(1, 0, 2)
    VWh[:, :, 64] = bf(1.0)
    VWh[:, :, 65] = bf(0.0)

    VF8h = np.zeros((128, 2, 32), dtype=f8)
    VF8h[:, :, 0] = f8(-(2.0**-5))

    in_maps = []
    for i in range(N_CORES):
        xs = xu[i * BS : (i + 1) * BS]               # (2048, 1024)
        xt = np.ascontiguousarray(xs.T)              # (1024, 2048) [k, b]
        A = xt.reshape(C, 128, MT, BT).transpose(1, 2, 0, 3)
        XUh = np.ascontiguousarray(A).astype(bf)
        in_maps.append({"XU": XUh, "VW": VWh, "VF8": VF8h})
    return in_maps


LAST_RESULTS = None  # stashed BassKernelResults (for test harness profiling)
TRACE = False


def kernel(x, vparam):
    global LAST_RESULTS
    nc = build_nc()
    in_maps = make_in_maps(x, vparam)
    res = run_bass_kernel_spmd(nc, in_maps, list(range(N_CORES)), trace=TRACE)
    LAST_RESULTS = res
    out = np.concatenate(
        [res.results[i]["OUT"].reshape(BS, 1) for i in range(N_CORES)], axis=0
    )
    return out.astype(np.float32)


# revision 6
# speedup vs baseline: 1.2079x; 1.0127x over previous
"""Trainium2 Bass kernel for nn_CrossProduct (factorization-machine cross term).

out_b = 0.5 * [ ||x_b V||^2 - sum_n w_n x_bn^2 ],  w = rowsum(V^2)

Math restructuring (v2): ship xu = 4*(x*sqrt(w)) in bf16. Then
  term2_b = sum_n xu_bn^2 / 32          (plain square-sum, no weights)
  term1   = sum_k (xu_b @ V')^2,  V' = v/(4*u*sqrt(2))
so the PE never streams a weighted x^2 matmul:
  - 8 bf16 matmuls/tile: psumA[64,512] += V'[c].T @ xu[c]
  - squares xu^2 -> fp8e4m3 on ACT(2 chunks)/DVE(4)/GpSimd(2)
  - 4 fp8 DoubleRow matmuls/tile (2 chunks per stream, const -2^-5 lhsT)
    accumulate -term2 into psumO[1,512]
  - ACT: sq = Square(psumA) -> bf16; 1 bf16 matmul adds ones.T @ sq
  - ACT copies psumO -> out row; single 8KB DMA at the end.

Distribution: pure data-parallel over batch across 8 NeuronCores
(2048 rows/core); vparam-derived constants replicated.
"""

import math
from contextlib import ExitStack

import ml_dtypes
import numpy as np

import concourse.bass as bass
import concourse.bacc as bacc
import concourse.mybir as mybir
import concourse.tile as tile
from concourse.bass_utils import run_bass_kernel_spmd

BF16 = mybir.dt.bfloat16
F32 = mybir.dt.float32
F8 = mybir.dt.float8e4

N_CORES = 8
B, XD, KD = 16384, 1024, 64
BS = B // N_CORES  # 2048 batch rows per core
C = XD // 128      # 8 contraction chunks of 128
MT = 4             # b-tiles per core
BT = BS // MT      # 512 batch cols per tile

DR = mybir.MatmulPerfMode.DoubleRow
SQUARE = mybir.ActivationFunctionType.Square


def _body(ctx, tc, OUT, XU, VW, VF8):
    nc = tc.nc
    const = ctx.enter_context(tc.tile_pool(name="const", bufs=1))
    xpool = ctx.enter_context(tc.tile_pool(name="xp", bufs=3))
    qpool = ctx.enter_context(tc.tile_pool(name="qp", bufs=2))
    sqpool = ctx.enter_context(tc.tile_pool(name="sqp", bufs=2))
    opool = ctx.enter_context(tc.tile_pool(name="op", bufs=1))
    psA = ctx.enter_context(tc.tile_pool(name="psA", bufs=2, space="PSUM"))
    psO = ctx.enter_context(tc.tile_pool(name="psO", bufs=2, space="PSUM"))

    # vw columns: [0:64]=V', 64=1.0 (sq-reduce weights), 65=0.0 (ACT bias)
    # Consts go FIRST on the same ring as x so their packets are not queued
    # behind 2MB of x data (the whole pipe waits on vw).
    vw = const.tile([128, C, 66], BF16)
    nc.sync.dma_start(vw[:], VW)
    vf8 = const.tile([128, 2, 32], F8)
    nc.sync.dma_start(vf8[:], VF8)
    outs = opool.tile([1, BS], F32)
    # one-time ACT touch of the vw DMA so later Square ops (which read the
    # bias column) carry only their data-dependency wait.
    actwarm = const.tile([128, 1], BF16)
    nc.scalar.copy(actwarm[:], vw[:, 0, 65:66])

    # PE p-state warmup: stream dummy matmuls on memset scratch while the
    # first x tile is still in flight, so real matmuls start at full clock.
    wpool = ctx.enter_context(tc.tile_pool(name="wp", bufs=1))
    psW = ctx.enter_context(tc.tile_pool(name="psW", bufs=1, space="PSUM"))
    scratch = wpool.tile([128, BT], BF16)
    nc.gpsimd.memset(scratch[:], 0)
    pw = psW.tile([64, BT], F32)
    for _ in range(7):
        nc.tensor.matmul(
            pw[:],
            scratch[:, 0:64],
            scratch[:],
            start=True,
            stop=True,
            tile_position=(0, 0),
        )

    for m in range(MT):
        xt = xpool.tile([128, C, BT], BF16)
        if m == 0:
            # fine-grained first-tile DMA so the PE starts ASAP
            for h in range(4):
                nc.sync.dma_start(xt[:, 2 * h : 2 * h + 2], XU[:, m, 2 * h : 2 * h + 2])
        else:
            nc.sync.dma_start(xt[:, 0:4], XU[:, m, 0:4])
            nc.sync.dma_start(xt[:, 4:8], XU[:, m, 4:8])

        # squares -> fp8, split across engines (chunk pairs align with the
        # DoubleRow rhs groups so each DR matmul has a single producer).
        # ACT is the fastest at fp8-out squares (DVE pays a 2x conversion
        # penalty, GpSimd is slow) so it takes the first two DR pairs.
        xq = qpool.tile([128, C, BT], F8)
        nc.scalar.activation(xq[:, 0:2], xt[:, 0:2], SQUARE, bias=vw[:, 0, 65:66])
        nc.scalar.activation(xq[:, 2:4], xt[:, 2:4], SQUARE, bias=vw[:, 0, 65:66])
        nc.vector.tensor_mul(xq[:, 4:6], xt[:, 4:6], xt[:, 4:6])
        nc.gpsimd.tensor_mul(xq[:, 6:8], xt[:, 6:8], xt[:, 6:8])

        pa = psA.tile([64, BT], F32)
        po = psO.tile([32, BT], F32)
        for c in range(C):
            nc.tensor.matmul(
                pa[:],
                vw[:, c, 0:64],
                xt[:, c],
                start=(c == 0),
                stop=(c == C - 1),
                tile_position=(0, 0),
            )
        for t in range(4):
            nc.tensor.matmul(
                po[:],
                vf8[:],
                xq[:, 2 * t : 2 * t + 2],
                start=(t == 0),
                stop=False,
                perf_mode=DR,
            )
        sq = sqpool.tile([64, BT], BF16)
        nc.scalar.activation(sq[:], pa[:], SQUARE, bias=vw[0:64, 0, 65:66])
        nc.tensor.matmul(
            po[0:1, :],
            vw[0:64, 0, 64:65],
            sq[:],
            start=False,
            stop=True,
        )
        # out-row copy on DVE (tensor_scalar is cheap there; keeps ACT free)
        nc.vector.tensor_scalar_add(outs[:, m * BT : (m + 1) * BT], po[0:1, :], 0.0)

    nc.sync.dma_start(OUT, outs[:])


_NC_CACHE = None


def build_nc():
    global _NC_CACHE
    if _NC_CACHE is not None:
        return _NC_CACHE
    nc = bacc.Bacc("TRN2", target_bir_lowering=False, debug=False)
    XU = nc.dram_tensor("XU", [128, MT, C, BT], BF16, kind="ExternalInput").ap()
    VW = nc.dram_tensor("VW", [128, C, 66], BF16, kind="ExternalInput").ap()
    VF8 = nc.dram_tensor("VF8", [128, 2, 32], F8, kind="ExternalInput").ap()
    OUT = nc.dram_tensor("OUT", [1, BS], F32, kind="ExternalOutput").ap()
    with tile.TileContext(nc) as tc:
        with ExitStack() as ctx:
            _body(ctx, tc, OUT, XU, VW, VF8)
    nc.compile()
    _NC_CACHE = nc
    return nc


def make_in_maps(x, vparam):
    bf = ml_dtypes.bfloat16
    f8 = ml_dtypes.float8_e4m3
    x = np.ascontiguousarray(x, dtype=np.float32)
    v = np.ascontiguousarray(vparam, dtype=np.float64)

    w = (v**2).sum(axis=1)                      # (1024,)
    u = np.sqrt(w)
    vs = (v / (u[:, None] * 4.0 * math.sqrt(2.0))).astype(np.float32).astype(bf)
    xu = (x.astype(np.float64) * (4.0 * u)[None, :]).astype(np.float32)

    VWh = np.empty((128, C, 66), dtype=bf)
    VWh[:, :, 0:64] = np.asarray(vs).reshape(C, 128, KD).transpose(1, 0, 2)
    VWh[:, :, 64] = bf(1.0)
    VWh[:, :, 65] = bf(0.0)

    VF8h = np.zeros((128, 2, 32), dtype=f8)
    VF8h[:, :, 0] = f8(-(2.0**-5))

    in_maps = []
    for i in range(N_CORES):
        xs = xu[i * BS : (i + 1) * BS]               # (2048, 1024)
        xt = np.ascontiguousarray(xs.T)              # (1024, 2048) [k, b]
        A = xt.reshape(C, 128, MT, BT).transpose(1, 2, 0, 3)
        XUh = np.ascontiguousarray(A).astype(bf)
        in_maps.append({"XU": XUh, "VW": VWh, "VF8": VF8h})
    return in_maps


LAST_RESULTS = None  # stashed BassKernelResults (for test harness profiling)
TRACE = False


def kernel(x, vparam):
    global LAST_RESULTS
    nc = build_nc()
    in_maps = make_in_maps(x, vparam)
    res = run_bass_kernel_spmd(nc, in_maps, list(range(N_CORES)), trace=TRACE)
    LAST_RESULTS = res
    out = np.concatenate(
        [res.results[i]["OUT"].reshape(BS, 1) for i in range(N_CORES)], axis=0
    )
    return out.astype(np.float32)
